# revision 7
# baseline (speedup 1.0000x reference)
"""F1-score (macro) kernel for Trainium2, 8 NeuronCores.

Data-parallel over rows (125000/core). Per tile of 2048 rows
([128p, TK=16, 128c], row = base + p*TK + k):

  - DVE : rowmax via pairwise-max tree. Step 1 reads the HIGH bf16 halves
          of the f32 tile (bitcast stride-2 view) -> truncated-bf16 max
          (trunc <= true max, so the argmax position always compares as
          "not less"). Steps 2..4 run at DVE 2x on packed bf16; final
          8-wide reduce emits f32.
  - DVE : oht_ck[p,c,k] = (c == y_true) in ck-layout - a single is_eq at
          DVE 2x (broadcasting t along the MIDDLE dim keeps every
          operand's last dim packed bf16).
  - anti split: first NDVE chunks as ONE sub-tile tensor_tensor is_lt on
          DVE ({0,1}); ~half the remaining on ACT via Sign ({-1,+1}!);
          one trailing chunk on GpSimd every other tile ({0,1}).
  - PE  : acc[bank] += oht_ck[:,:,k]^T @ anti[:,k,:] (bf16, 4 PSUM banks:
          banks 0/1 collect {0,1}-chunks, banks 2/3 the {-1,+1} chunks).

Host: with per-group supports S01/S23 (row->engine map is static):
  pred01 = S01 - (bank0+bank1),  pred23 = (S23 - (bank2+bank3)) / 2
  cm = pred01 + pred23;  macro-F1 epilogue on [128,128].

bf16 truncation ties perturb F1 by ~2.7e-4 (measured in numpy on the
actual inputs), far under the 2e-2 gate.
"""

import sys
import time

if "/opt/trn_rl_repo" not in sys.path:
    sys.path.insert(0, "/opt/trn_rl_repo")

import numpy as np

import concourse.bacc as bacc
import concourse.mybir as mybir
import concourse.tile as tile
from concourse import bass_utils

C = 128
N = 1_000_000
NCORES = 8
R = N // NCORES          # 125000 rows per core
TK = 16                  # chunks (of 128 rows) per tile
TR = 128 * TK            # 2048 rows per tile
NT = R // TR             # 61 tiles
TAIL = R - NT * TR       # 72 rows
EPS = 1e-12

NDVE = 3                 # max leading anti chunks per tile on DVE ({0,1})


def _gs_chunks(i):
    """GpSimd-owned trailing anti chunks ({0,1}) for tile i."""
    return (15,)


def _ndve(i):
    """Leading anti chunks on DVE for tile i."""
    return 3 if i % 2 == 0 else 2


def _schedule():
    """(i, k) -> (engine, bank); engine in {'dve','act','gs'}."""
    sched = {}
    nb01 = 0
    nb23 = 0
    for i in range(NT):
        gs = _gs_chunks(i)
        nd = _ndve(i)
        for k in range(TK):
            if k < nd:
                sched[(i, k)] = ("dve", nb01 % 4)
                nb01 += 1
            elif k in gs:
                sched[(i, k)] = ("gs", nb01 % 4)
                nb01 += 1
            else:
                sched[(i, k)] = ("act", 4 + nb23 % 4)
                nb23 += 1
    return sched


_SCHED = _schedule()
# last matmul per bank (tail is emitted first, so bank 0 also ends in-loop)
_LAST = {}
for (_i, _k), (_e, _b) in _SCHED.items():
    _LAST[_b] = max(_LAST.get(_b, (-1, -1)), (_i, _k))

_CACHE = {}


def _build():
    f32 = mybir.dt.float32
    bf16 = mybir.dt.bfloat16
    Alu = mybir.AluOpType
    Act = mybir.ActivationFunctionType

    nc = bacc.Bacc("TRN2", target_bir_lowering=False, debug=False,
                   num_devices=NCORES)
    yp = nc.dram_tensor("yp", [R, C], f32, kind="ExternalInput")
    yt = nc.dram_tensor("yt", [R], bf16, kind="ExternalInput")
    cm4 = nc.dram_tensor("cm4", [C, 8, C], f32, kind="ExternalOutput")

    with tile.TileContext(nc) as tc:
        with (
            tc.tile_pool(name="const", bufs=1) as cpool,
            tc.tile_pool(name="xin", bufs=6) as xpool,
            tc.tile_pool(name="oh", bufs=5) as ohpool,
            tc.tile_pool(name="an", bufs=5) as anpool,
            tc.tile_pool(name="small", bufs=6) as spool,
            tc.tile_pool(name="psum", bufs=1, space="PSUM") as psum,
        ):
            iota_i = cpool.tile([128, C], mybir.dt.int32)
            nc.gpsimd.iota(iota_i[:], pattern=[[1, C]], base=0,
                           channel_multiplier=0)
            iota_bf = cpool.tile([128, C], bf16)
            nc.vector.tensor_copy(iota_bf[:], iota_i[:])
            # iota_ck[p, c, k] = c  (constant along inner k)
            iota_ck = cpool.tile([128, C, TK], bf16)
            nc.vector.tensor_copy(
                iota_ck[:], iota_bf[:, :, None].broadcast_to([128, C, TK])
            )

            accs = [
                psum.tile([C, C], f32, tag=f"acc{j}", name=f"acc{j}")
                for j in range(8)
            ]
            started = [False] * 8

            def mm(bank, lhsT, rhs, is_last=False):
                nc.tensor.matmul(
                    accs[bank][:], lhsT, rhs,
                    start=not started[bank], stop=is_last,
                )
                started[bank] = True

            def emit_tile(i):
                base = i * TR
                x = xpool.tile([128, TK, C], f32, tag="x", name="x")
                nc.sync.dma_start(
                    x[:],
                    yp.ap()[base : base + TR, :].rearrange(
                        "(p k) c -> p k c", k=TK
                    ),
                )
                t = spool.tile([128, TK], bf16, tag="t", name="t")
                nc.sync.dma_start(
                    t[:],
                    yt.ap()[base : base + TR].rearrange("(p k) -> p k", k=TK),
                )

                # one-hot(true) in ck layout: single 2x is_eq
                oht = ohpool.tile([128, C, TK], bf16, tag="oht", name="oht")
                nc.vector.tensor_tensor(
                    oht[:], iota_ck[:],
                    t[:, None, :].broadcast_to([128, C, TK]),
                    op=Alu.is_equal,
                )

                # exact f32 rowmax (reduce is input-count-bound; no tree
                # variant beats it on this hardware)
                rmax = spool.tile([128, TK], f32, tag="rmax", name="rmax")
                nc.vector.tensor_reduce(
                    rmax[:], x[:], axis=mybir.AxisListType.X, op=Alu.max
                )

                # anti chunks
                anti = anpool.tile([128, TK, C], bf16, tag="anti",
                                   name="anti")
                nd = _ndve(i)
                nc.vector.tensor_tensor(
                    anti[:, 0:nd, :], x[:, 0:nd, :],
                    rmax[:, 0:nd, None].broadcast_to([128, nd, C]),
                    op=Alu.is_lt,
                )
                gs = _gs_chunks(i)
                for k in range(nd, TK):
                    if k in gs:
                        nc.gpsimd.tensor_scalar(
                            anti[:, k, :], x[:, k, :], rmax[:, k : k + 1],
                            None, op0=Alu.is_lt,
                        )
                    else:
                        # sign(rmax - x) in {0,1}: x <= rmax always (exact)
                        nc.scalar.activation(
                            anti[:, k, :], x[:, k, :], Act.Sign,
                            bias=rmax[:, k : k + 1], scale=-1.0,
                        )

                for k in range(TK):
                    eng, bank = _SCHED[(i, k)]
                    mm(bank, oht[:, :, k], anti[:, k, :],
                       is_last=(_LAST[bank] == (i, k)))

            # tail rows (72) first: keeps the odd chain off the
            # critical path; its matmul opens the bank-0 accumulation
            base = NT * TR
            xt = xpool.tile([TAIL, 1, C], f32, tag="xtail", name="xt")
            nc.sync.dma_start(
                xt[:],
                yp.ap()[base : R, :].rearrange("(p k) c -> p k c", k=1),
            )
            tt = spool.tile([TAIL, 1], bf16, tag="ttail", name="tt")
            nc.sync.dma_start(
                tt[:], yt.ap()[base : R].rearrange("(p k) -> p k", k=1)
            )
            tt_f = spool.tile([TAIL, 1], f32, tag="ttailf", name="tt_f")
            nc.vector.tensor_copy(tt_f[:], tt[:])
            rmax_t = spool.tile([TAIL, 1], f32, tag="rmaxtail", name="rmax_t")
            nc.vector.tensor_reduce(
                rmax_t[:], xt[:], axis=mybir.AxisListType.X, op=Alu.max
            )
            anti_t = anpool.tile([TAIL, C], bf16, tag="antitail",
                                 name="anti_t")
            oht_t = ohpool.tile([TAIL, C], bf16, tag="ohttail", name="oht_t")
            nc.vector.tensor_scalar(
                anti_t[:], xt[:, 0, :], rmax_t[:], None, op0=Alu.is_lt
            )
            nc.vector.tensor_scalar(
                oht_t[:], iota_bf[:TAIL, :], tt_f[:], None, op0=Alu.is_equal
            )
            nc.tensor.matmul(
                accs[0][:], oht_t[:], anti_t[:], start=True, stop=False
            )
            started[0] = True

            for i in range(NT):
                emit_tile(i)

            out_sb = cpool.tile([C, 8, C], f32)
            for j in range(8):
                nc.scalar.copy(out_sb[:, j, :], accs[j][:])
            nc.sync.dma_start(cm4.ap()[:], out_sb[:])

    nc.compile()
    return nc


def _group01_mask():
    """Per-core mask over R rows: True if the row's anti chunk has {0,1}
    semantics (DVE/GpSimd/tail), False for ACT ({-1,+1})."""
    r = np.arange(R)
    i = r // TR
    k = r % TK
    m = i == NT  # tail rows
    for it in range(NT):
        sel = i == it
        m |= sel & (k < _ndve(it))
        gs = _gs_chunks(it)
        if gs:
            m |= sel & np.isin(k, gs)
    return m


_G01 = _group01_mask()


def _get_nc():
    if "nc" not in _CACHE:
        _CACHE["nc"] = _build()
    return _CACHE["nc"]


def _run(y_pred, y_true, trace=False):
    import ml_dtypes

    nc = _get_nc()
    y_pred = np.ascontiguousarray(np.asarray(y_pred, dtype=np.float32))
    yt_i = np.asarray(y_true).astype(np.int64)
    yt_bf = yt_i.astype(ml_dtypes.bfloat16)
    in_maps = [
        {
            "yp": y_pred[c * R : (c + 1) * R],
            "yt": np.ascontiguousarray(yt_bf[c * R : (c + 1) * R]),
        }
        for c in range(NCORES)
    ]
    res = None
    for attempt in range(3):
        try:
            res = bass_utils.run_bass_kernel_spmd(
                nc, in_maps, core_ids=list(range(NCORES)), trace=trace
            )
            break
        except Exception:
            if attempt == 2:
                raise
            time.sleep(2.0)
    cm_dev = np.zeros((C, C), dtype=np.float64)
    for r in res.results:
        cm_dev += r["cm4"].astype(np.float64).sum(axis=1)
    support = np.bincount(yt_i, minlength=C).astype(np.float64)
    cm = support[:, None] - cm_dev
    diag = np.diagonal(cm)
    precision = diag / (cm.sum(axis=1) + EPS)
    recall = diag / (cm.sum(axis=0) + EPS)
    f1 = 2.0 * precision * recall / (precision + recall + EPS)
    return np.float32(f1.mean()), res


def kernel(y_pred, y_true):
    out, _ = _run(y_pred, y_true, trace=False)
    return out


# revision 9
# speedup vs baseline: 1.0218x; 1.0218x over previous
"""F1-score (macro) kernel for Trainium2, 8 NeuronCores.

Data-parallel over rows (125000/core). Per tile of 2048 rows
([128p, TK=16, 128c], row = base + p*TK + k):

  - DVE : rowmax via pairwise-max tree. Step 1 reads the HIGH bf16 halves
          of the f32 tile (bitcast stride-2 view) -> truncated-bf16 max
          (trunc <= true max, so the argmax position always compares as
          "not less"). Steps 2..4 run at DVE 2x on packed bf16; final
          8-wide reduce emits f32.
  - DVE : oht_ck[p,c,k] = (c == y_true) in ck-layout - a single is_eq at
          DVE 2x (broadcasting t along the MIDDLE dim keeps every
          operand's last dim packed bf16).
  - anti split: first NDVE chunks as ONE sub-tile tensor_tensor is_lt on
          DVE ({0,1}); ~half the remaining on ACT via Sign ({-1,+1}!);
          one trailing chunk on GpSimd every other tile ({0,1}).
  - PE  : acc[bank] += oht_ck[:,:,k]^T @ anti[:,k,:] (bf16, 4 PSUM banks:
          banks 0/1 collect {0,1}-chunks, banks 2/3 the {-1,+1} chunks).

Host: with per-group supports S01/S23 (row->engine map is static):
  pred01 = S01 - (bank0+bank1),  pred23 = (S23 - (bank2+bank3)) / 2
  cm = pred01 + pred23;  macro-F1 epilogue on [128,128].

bf16 truncation ties perturb F1 by ~2.7e-4 (measured in numpy on the
actual inputs), far under the 2e-2 gate.
"""

import sys
import time

if "/opt/trn_rl_repo" not in sys.path:
    sys.path.insert(0, "/opt/trn_rl_repo")

import numpy as np

import concourse.bacc as bacc
import concourse.mybir as mybir
import concourse.tile as tile
from concourse import bass_utils

C = 128
N = 1_000_000
NCORES = 8
R = N // NCORES          # 125000 rows per core
TK = 16                  # chunks (of 128 rows) per tile
TR = 128 * TK            # 2048 rows per tile
NT = R // TR             # 61 tiles
TAIL = R - NT * TR       # 72 rows
EPS = 1e-12

NDVE = 3                 # max leading anti chunks per tile on DVE ({0,1})


def _gs_chunks(i):
    """GpSimd-owned trailing anti chunks ({0,1}) for tile i."""
    return () if i == NT - 1 else (15,)


def _ndve(i):
    """Leading anti chunks on DVE for tile i."""
    if i == NT - 1:
        return TK  # last tile all-DVE: no ACT/GpSimd drain at the end
    return 3 if i % 2 == 0 else 2


def _schedule():
    """(i, k) -> (engine, bank); engine in {'dve','act','gs'}."""
    sched = {}
    nb01 = 0
    nb23 = 0
    for i in range(NT):
        gs = _gs_chunks(i)
        nd = _ndve(i)
        for k in range(TK):
            if k < nd:
                sched[(i, k)] = ("dve", nb01 % 4)
                nb01 += 1
            elif k in gs:
                sched[(i, k)] = ("gs", nb01 % 4)
                nb01 += 1
            else:
                sched[(i, k)] = ("act", 4 + nb23 % 4)
                nb23 += 1
    return sched


_SCHED = _schedule()
# last matmul per bank (tail is emitted first, so bank 0 also ends in-loop)
_LAST = {}
for (_i, _k), (_e, _b) in _SCHED.items():
    _LAST[_b] = max(_LAST.get(_b, (-1, -1)), (_i, _k))

_CACHE = {}


def _build():
    f32 = mybir.dt.float32
    bf16 = mybir.dt.bfloat16
    Alu = mybir.AluOpType
    Act = mybir.ActivationFunctionType

    nc = bacc.Bacc("TRN2", target_bir_lowering=False, debug=False,
                   num_devices=NCORES)
    yp = nc.dram_tensor("yp", [R, C], f32, kind="ExternalInput")
    yt = nc.dram_tensor("yt", [R], bf16, kind="ExternalInput")
    cm4 = nc.dram_tensor("cm4", [C, 8, C], f32, kind="ExternalOutput")

    with tile.TileContext(nc) as tc:
        with (
            tc.tile_pool(name="const", bufs=1) as cpool,
            tc.tile_pool(name="xin", bufs=6) as xpool,
            tc.tile_pool(name="oh", bufs=5) as ohpool,
            tc.tile_pool(name="an", bufs=5) as anpool,
            tc.tile_pool(name="small", bufs=6) as spool,
            tc.tile_pool(name="psum", bufs=1, space="PSUM") as psum,
        ):
            def load_x(i):
                base = i * TR
                x = xpool.tile([128, TK, C], f32, tag="x", name="x")
                nc.sync.dma_start(
                    x[:],
                    yp.ap()[base : base + TR, :].rearrange(
                        "(p k) c -> p k c", k=TK
                    ),
                )
                return x

            # issue the first x loads before anything else so DVE's first
            # reduce isn't waiting on DMA at startup
            xq = [load_x(i) for i in range(3)]

            iota_i = cpool.tile([128, C], mybir.dt.int32)
            nc.gpsimd.iota(iota_i[:], pattern=[[1, C]], base=0,
                           channel_multiplier=0)
            iota_bf = cpool.tile([128, C], bf16)
            nc.vector.tensor_copy(iota_bf[:], iota_i[:])
            # iota_ck[p, c, k] = c  (constant along inner k)
            iota_ck = cpool.tile([128, C, TK], bf16)
            nc.vector.tensor_copy(
                iota_ck[:], iota_bf[:, :, None].broadcast_to([128, C, TK])
            )

            accs = [
                psum.tile([C, C], f32, tag=f"acc{j}", name=f"acc{j}")
                for j in range(8)
            ]
            started = [False] * 8

            def mm(bank, lhsT, rhs, is_last=False):
                nc.tensor.matmul(
                    accs[bank][:], lhsT, rhs,
                    start=not started[bank], stop=is_last,
                )
                started[bank] = True

            def emit_tile(i, x):
                base = i * TR
                t = spool.tile([128, TK], bf16, tag="t", name="t")
                nc.sync.dma_start(
                    t[:],
                    yt.ap()[base : base + TR].rearrange("(p k) -> p k", k=TK),
                )

                # one-hot(true) in ck layout: single 2x is_eq
                oht = ohpool.tile([128, C, TK], bf16, tag="oht", name="oht")
                nc.vector.tensor_tensor(
                    oht[:], iota_ck[:],
                    t[:, None, :].broadcast_to([128, C, TK]),
                    op=Alu.is_equal,
                )

                # exact f32 rowmax (reduce is input-count-bound; no tree
                # variant beats it on this hardware)
                rmax = spool.tile([128, TK], f32, tag="rmax", name="rmax")
                nc.vector.tensor_reduce(
                    rmax[:], x[:], axis=mybir.AxisListType.X, op=Alu.max
                )

                # anti chunks
                anti = anpool.tile([128, TK, C], bf16, tag="anti",
                                   name="anti")
                nd = _ndve(i)
                nc.vector.tensor_tensor(
                    anti[:, 0:nd, :], x[:, 0:nd, :],
                    rmax[:, 0:nd, None].broadcast_to([128, nd, C]),
                    op=Alu.is_lt,
                )
                gs = _gs_chunks(i)
                for k in range(nd, TK):
                    if k in gs:
                        nc.gpsimd.tensor_scalar(
                            anti[:, k, :], x[:, k, :], rmax[:, k : k + 1],
                            None, op0=Alu.is_lt,
                        )
                    else:
                        # sign(rmax - x) in {0,1}: x <= rmax always (exact)
                        nc.scalar.activation(
                            anti[:, k, :], x[:, k, :], Act.Sign,
                            bias=rmax[:, k : k + 1], scale=-1.0,
                        )

                for k in range(TK):
                    eng, bank = _SCHED[(i, k)]
                    mm(bank, oht[:, :, k], anti[:, k, :],
                       is_last=(_LAST[bank] == (i, k)))

            # tail rows (72) first: keeps the odd chain off the
            # critical path; its matmul opens the bank-0 accumulation
            base = NT * TR
            xt = xpool.tile([TAIL, 1, C], f32, tag="xtail", name="xt")
            nc.sync.dma_start(
                xt[:],
                yp.ap()[base : R, :].rearrange("(p k) c -> p k c", k=1),
            )
            tt = spool.tile([TAIL, 1], bf16, tag="ttail", name="tt")
            nc.sync.dma_start(
                tt[:], yt.ap()[base : R].rearrange("(p k) -> p k", k=1)
            )
            tt_f = spool.tile([TAIL, 1], f32, tag="ttailf", name="tt_f")
            nc.vector.tensor_copy(tt_f[:], tt[:])
            rmax_t = spool.tile([TAIL, 1], f32, tag="rmaxtail", name="rmax_t")
            nc.vector.tensor_reduce(
                rmax_t[:], xt[:], axis=mybir.AxisListType.X, op=Alu.max
            )
            anti_t = anpool.tile([TAIL, C], bf16, tag="antitail",
                                 name="anti_t")
            oht_t = ohpool.tile([TAIL, C], bf16, tag="ohttail", name="oht_t")
            nc.vector.tensor_scalar(
                anti_t[:], xt[:, 0, :], rmax_t[:], None, op0=Alu.is_lt
            )
            nc.vector.tensor_scalar(
                oht_t[:], iota_bf[:TAIL, :], tt_f[:], None, op0=Alu.is_equal
            )
            nc.tensor.matmul(
                accs[0][:], oht_t[:], anti_t[:], start=True, stop=False
            )
            started[0] = True

            for i in range(NT):
                x = xq[i] if i < len(xq) else load_x(i)
                emit_tile(i, x)

            out_sb = cpool.tile([C, 8, C], f32)
            for j in range(8):
                nc.scalar.copy(out_sb[:, j, :], accs[j][:])
            nc.sync.dma_start(cm4.ap()[:], out_sb[:])

    nc.compile()
    return nc


def _group01_mask():
    """Per-core mask over R rows: True if the row's anti chunk has {0,1}
    semantics (DVE/GpSimd/tail), False for ACT ({-1,+1})."""
    r = np.arange(R)
    i = r // TR
    k = r % TK
    m = i == NT  # tail rows
    for it in range(NT):
        sel = i == it
        m |= sel & (k < _ndve(it))
        gs = _gs_chunks(it)
        if gs:
            m |= sel & np.isin(k, gs)
    return m


_G01 = _group01_mask()


def _get_nc():
    if "nc" not in _CACHE:
        _CACHE["nc"] = _build()
    return _CACHE["nc"]


def _run(y_pred, y_true, trace=False):
    import ml_dtypes

    nc = _get_nc()
    y_pred = np.ascontiguousarray(np.asarray(y_pred, dtype=np.float32))
    yt_i = np.asarray(y_true).astype(np.int64)
    yt_bf = yt_i.astype(ml_dtypes.bfloat16)
    in_maps = [
        {
            "yp": y_pred[c * R : (c + 1) * R],
            "yt": np.ascontiguousarray(yt_bf[c * R : (c + 1) * R]),
        }
        for c in range(NCORES)
    ]
    res = None
    for attempt in range(3):
        try:
            res = bass_utils.run_bass_kernel_spmd(
                nc, in_maps, core_ids=list(range(NCORES)), trace=trace
            )
            break
        except Exception:
            if attempt == 2:
                raise
            time.sleep(2.0)
    cm_dev = np.zeros((C, C), dtype=np.float64)
    for r in res.results:
        cm_dev += r["cm4"].astype(np.float64).sum(axis=1)
    support = np.bincount(yt_i, minlength=C).astype(np.float64)
    cm = support[:, None] - cm_dev
    diag = np.diagonal(cm)
    precision = diag / (cm.sum(axis=1) + EPS)
    recall = diag / (cm.sum(axis=0) + EPS)
    f1 = 2.0 * precision * recall / (precision + recall + EPS)
    return np.float32(f1.mean()), res


def kernel(y_pred, y_true):
    out, _ = _run(y_pred, y_true, trace=False)
    return out
